# revision 1
# baseline (speedup 1.0000x reference)
"""J-regularized cross-entropy loss on 8 Trainium2 cores.

Math per core (2 batches, N=262144 pixels, C=8):
  S[b,k,ci]   = sum_p pred[b,ci,p] * (target[b,p]==k)   (8x8 per batch)
  lse[b,p]    = log sum_c exp(pred[b,c,p])
  host: M = S^T/n, jl = mean_b -sum_{ci!=ck} log(.5+.5*(diag-M)),
        ce = (sum lse - sum_b tr S)/(B*N), out = jl + ce.

Engine split (exp over 8N elements at ACT's fixed 1 elem/cycle/lane is
the wall): pred arrives pixel-major (t,c), column-split into an fp8e3
part (ACT exp reads fp8 at full rate) and a bf16 part whose exp runs on
the DVE as a Schraudolph bitcast-exp (one 4x tensor_scalar into an
int16 view of the bf16 exp tile; bias calibrated for mean ratio 1).
One-hot weights: first OHH d-groups DMA'd from host (fp8e4), the rest
built on DVE (is_equal, 4x). Class sums per HALF chunk: bf16
tensor_tensor tree L1+L2 on DVE, L3 on GpSimd (last chunk on DVE), Ln
with accum_out on ACT; the t<512 half depends only on the ACT exp so
trees pipeline inside the exp stream. A manual LoadActFuncSet of
natural_log_exp_and_others removes all table switches. S accumulates
in PSUM via mixed-dtype matmuls.
"""

import numpy as np
import ml_dtypes

import concourse.bacc as bacc
import concourse.mybir as mybir
import concourse.tile as tile
from concourse import bass_utils

N_CORES = 8
B, C, H, W = 16, 8, 512, 512
N = H * W                 # pixels per batch
P = 128                   # SBUF partitions
COLS = N // P             # 2048 pixel-columns per batch
F = 1024                  # pixel-columns per chunk
CH = COLS // F            # chunks per batch
BPC = B // N_CORES        # batches per core
G = 16                    # pixel-columns per matmul group (16*8=128)
NDG = F // G              # matmul d-groups per chunk (64)

ND8 = 40                  # fp8 d-groups per chunk (ACT exp share)
NDB = NDG - ND8           # bf16 d-groups per chunk (DVE schraudolph)
A8 = ND8 * 128            # fp8 free size per chunk (5376)
AB = NDB * 128            # bf16 free size per chunk (2816)
T8 = A8 // C              # fp8 pixel-cols per chunk (672)
HF = F // 2               # half-chunk pixel-cols (512)
EH0 = HF * C              # exp free for half 0 (4096, all fp8)

OHH = 48                  # host one-hot d-groups per chunk (fp8e4)
OHD = NDG - OHH           # device one-hot d-groups
TOD = OHD * G             # device-oh pixel-cols per chunk (512)

# packed per-chunk input row (bytes per partition):
#   [pred8 | predb | ohh | tgt]
B_P8 = A8
B_PB = AB * 2
B_OH = OHH * 128
O_PB = B_P8
O_OH = O_PB + B_PB
B_PK = O_OH + B_OH

LOG2E = 1.4426950408889634
SCHRAU_A = 128.0 * LOG2E
SCHRAU_B = 16256.0 - 7.368   # mean multiplicative error centered at 1
LN2 = 0.6931471805599453
# inverse trick: ln(x) ~ (bits_bf16(x) - 16256 + 7.334) * ln2/128
DVELN_S1 = -(16256.0 - 7.334)
DVELN_S2 = LN2 / 128.0
ACT_SET_NL_EXP = 6           # natural_log_exp_and_others

TRACE = False
LAST_EXEC_NS = None
LAST_TRACE = None

_BF16 = mybir.dt.bfloat16
_F32 = mybir.dt.float32
_F8E3 = mybir.dt.float8e3
_F8E4 = mybir.dt.float8e4
_I16 = mybir.dt.int16

_nc_cache = None


def _build_nc():
    nc = bacc.Bacc("TRN2", target_bir_lowering=False, debug=False,
                   num_devices=N_CORES)
    pk_d = nc.dram_tensor("packed", (BPC, CH, P, B_PK), mybir.dt.uint8,
                          kind="ExternalInput")
    tgt_d = nc.dram_tensor("tgt", (P, BPC * CH * TOD), _BF16,
                           kind="ExternalInput")
    out_d = nc.dram_tensor("out", (P, 2 * C * G + 2 * BPC * CH + 1), _F32,
                           kind="ExternalOutput")

    NCH = BPC * CH
    with tile.TileContext(nc) as tc:
        # combined exp+ln table load up front; overlaps input DMA
        nc.scalar.add_instruction(mybir.InstLoadActFuncSet(
            name=nc.get_next_instruction_name(),
            act_func_set_id=ACT_SET_NL_EXP, ins=[], outs=[]))
        with (
            tc.tile_pool(name="pk", bufs=4) as pk_pool,
            tc.tile_pool(name="ohd", bufs=4) as ohd_pool,
            tc.tile_pool(name="exp", bufs=3) as exp_pool,
            tc.tile_pool(name="small", bufs=2) as small_pool,
            tc.tile_pool(name="acc", bufs=1) as acc_pool,
            tc.tile_pool(name="psum", bufs=2, space="PSUM") as psum_pool,
        ):
            out_sb = acc_pool.tile([P, 2 * C * G + 2 * NCH + 1], _F32,
                                   name="out_sb")
            lse_acc = out_sb[:, 2 * C * G:]

            pk_t, ohd_t, exp_t = {}, {}, {}

            def views(ci):
                pk = pk_t[ci]
                p8 = pk[:, :B_P8].bitcast(_F8E3)
                pb = pk[:, O_PB:O_OH].bitcast(_BF16)
                oh = pk[:, O_OH:].bitcast(_F8E4)
                return p8, pb, oh

            def dma_piece(ci, lo, hi):
                b, ch = divmod(ci, CH)
                if ci not in pk_t:
                    pk_t[ci] = pk_pool.tile([P, B_PK], mybir.dt.uint8,
                                            tag="pk", name="pk")
                nc.sync.dma_start(pk_t[ci][:, lo:hi],
                                  pk_d[b, ch, :, lo:hi])

            def dma_all(tgt_all):
                # one-chunk lookahead: next chunk's exp bytes jump ahead
                # of the current chunk's bulk (pb/oh) pieces; all targets
                # land in one tiny early DMA so one-hot fills the DVE head
                dma_piece(0, 0, 2048)
                nc.sync.dma_start(tgt_all[:, :], tgt_d[:, :])
                dma_piece(0, 2048, EH0)
                dma_piece(0, EH0, B_P8)
                dma_piece(1, 0, B_P8)
                dma_piece(0, B_P8, B_PK)
                dma_piece(2, 0, B_P8)
                dma_piece(1, B_P8, B_PK)
                dma_piece(3, 0, B_P8)
                dma_piece(2, B_P8, B_PK)
                dma_piece(3, B_P8, B_PK)

            def act_exp(ci, half):
                p8 = views(ci)[0]
                if half == 0:
                    exp_t[ci] = exp_pool.tile([P, F * C], _BF16, tag="e",
                                              name="e")
                    if ci == 0:
                        nc.scalar.activation(
                            exp_t[ci][:, :2048], p8[:, :2048],
                            mybir.ActivationFunctionType.Exp)
                        nc.scalar.activation(
                            exp_t[ci][:, 2048:EH0], p8[:, 2048:EH0],
                            mybir.ActivationFunctionType.Exp)
                    else:
                        nc.scalar.activation(
                            exp_t[ci][:, :EH0], p8[:, :EH0],
                            mybir.ActivationFunctionType.Exp)
                else:
                    nc.scalar.activation(
                        exp_t[ci][:, EH0:A8], p8[:, EH0:],
                        mybir.ActivationFunctionType.Exp)

            def dve_schrau(ci):
                pb = views(ci)[1]
                nc.vector.tensor_scalar(
                    exp_t[ci][:, A8:].bitcast(_I16), pb,
                    SCHRAU_A, SCHRAU_B,
                    mybir.AluOpType.mult, mybir.AluOpType.add)

            def dve_oh(ci, tgt_all):
                ohd_t[ci] = ohd_pool.tile([P, OHD * C * G], _BF16,
                                          tag="ohd", name="ohd")
                oh4 = ohd_t[ci][:, :].rearrange("p (d k g) -> p d k g",
                                                k=C, g=G)
                tgt3 = tgt_all[:, ci * TOD:(ci + 1) * TOD].rearrange(
                    "p (d g) -> p d g", g=G)
                for k in range(C):
                    nc.vector.tensor_scalar(
                        oh4[:, :, k, :], tgt3,
                        float(k), None, mybir.AluOpType.is_equal)

            def matmuls(ci, psum_t):
                b, ch = divmod(ci, CH)
                p8, pb, ohh = views(ci)
                for d in range(NDG):
                    if d < OHH:
                        lhsT = ohh[:, d * 128:(d + 1) * 128]
                    else:
                        dd = d - OHH
                        lhsT = ohd_t[ci][:, dd * 128:(dd + 1) * 128]
                    if d < ND8:
                        rhs = p8[:, d * 128:(d + 1) * 128]
                    else:
                        dd = d - ND8
                        rhs = pb[:, dd * 128:(dd + 1) * 128]
                    nc.tensor.matmul(
                        psum_t[:, :], lhsT, rhs,
                        start=(ch == 0 and d == 0),
                        stop=(ch == CH - 1 and d == NDG - 1),
                    )

            def tree12(ci, t0, t1):
                w = t1 - t0
                e3 = exp_t[ci][:, t0 * C:t1 * C]\
                    .rearrange("p (t c) -> p t c", c=C)
                tmp1 = small_pool.tile([P, w, 4], _BF16, tag=f"tmp1{w}",
                                       name="tmp1")
                nc.vector.tensor_add(tmp1[:, :, :], e3[:, :, 0:4],
                                     e3[:, :, 4:8])
                tmp2 = small_pool.tile([P, w, 2], _BF16, tag=f"tmp2{w}",
                                       name="tmp2")
                nc.vector.tensor_add(tmp2[:, :, :], tmp1[:, :, 0:2],
                                     tmp1[:, :, 2:4])
                return tmp2

            def tree3(tmp2, w):
                sume = small_pool.tile([P, w], _BF16, tag=f"sume{w}",
                                      name="sume")
                nc.vector.tensor_add(sume[:, :], tmp2[:, :, 0],
                                     tmp2[:, :, 1])
                return sume

            def act_ln(col, sume, w):
                lnsc = small_pool.tile([P, w], _BF16, tag=f"lnsc{w}",
                                       name="lnsc")
                nc.scalar.activation(
                    lnsc[:, :], sume[:, :],
                    mybir.ActivationFunctionType.Ln,
                    accum_out=lse_acc[:, col:col + 1],
                )

            def dve_ln(col, sume, w):
                # raw sum of bf16 bit patterns; host scales by ln2/128
                lnsc = small_pool.tile([P, w], _BF16, tag=f"lnsd{w}",
                                       name="lnsd")
                nc.vector.tensor_scalar(
                    lnsc[:, :], sume[:, :].bitcast(_I16),
                    DVELN_S1, None,
                    mybir.AluOpType.add, mybir.AluOpType.add,
                    accum_out=lse_acc[:, col:col + 1],
                )

            tgt_all = small_pool.tile([P, NCH * TOD], _BF16,
                                      name="tgt_all", bufs=1)
            dma_all(tgt_all)
            psums = {}
            for b in range(BPC):
                psums[b] = psum_pool.tile([P, C * G], _F32, tag="ps",
                                          name="ps")

            pend = None          # (col, sume, w) from previous stage
            for ci2 in range(NCH):
                dve_oh(ci2, tgt_all)
            for ci in range(NCH):
                b = ci // CH
                last = ci == NCH - 1
                act_exp(ci, 0)
                dve_schrau(ci)
                if pend is not None:
                    act_ln(*pend)
                    pend = None
                act_exp(ci, 1)
                s0 = tree3(tree12(ci, 0, HF), HF)
                act_ln(2 * ci, s0, HF)
                if not last:
                    s1 = tree3(tree12(ci, HF, F), HF)
                    pend = (2 * ci + 1, s1, HF)
                matmuls(ci, psums[b])
                if ci == CH - 1:
                    nc.vector.tensor_copy(out_sb[:, :C * G],
                                          psums[0][:, :])
            # last chunk second half in quarters for a short tail
            Q = HF // 2
            sq0 = tree3(tree12(NCH - 1, HF, HF + Q), Q)
            act_ln(2 * NCH - 1, sq0, Q)
            nc.vector.tensor_copy(out_sb[:, C * G:2 * C * G],
                                  psums[BPC - 1][:, :])
            nc.sync.dma_start(out_d[:, :2 * C * G], out_sb[:, :2 * C * G])
            sq1 = tree3(tree12(NCH - 1, HF + Q, F), Q)
            act_ln(2 * NCH, sq1, Q)
            nc.sync.dma_start(out_d[:, 2 * C * G:], out_sb[:, 2 * C * G:],
                              single_packet=True)

    nc.compile()
    return nc


def kernel(pred, target):
    global LAST_EXEC_NS, LAST_TRACE, _nc_cache
    pred = np.asarray(pred)
    target = np.asarray(target)

    if _nc_cache is None:
        _nc_cache = _build_nc()
    nc = _nc_cache

    # pixel-major device layout: (b, ch, p, t, c); split t into fp8/bf16
    predv = np.asarray(pred, dtype=np.float32).reshape(B, C, P, CH, F)
    tgtf = target.reshape(B, P, CH, NDG, G).transpose(0, 2, 1, 3, 4)
    # tgtf[b, ch, p, d, g]
    in_maps = []
    for core in range(N_CORES):
        bs = slice(core * BPC, (core + 1) * BPC)
        pc = predv[bs].transpose(0, 3, 2, 4, 1)          # (BPC, CH, P, F, C)
        pc = np.ascontiguousarray(pc)
        p8 = np.ascontiguousarray(pc[:, :, :, :T8, :]).reshape(BPC, CH, P, A8)
        pb = np.ascontiguousarray(pc[:, :, :, T8:, :]).reshape(BPC, CH, P, AB)
        p8 = p8.astype(ml_dtypes.float8_e3m4)
        pb = pb.astype(ml_dtypes.bfloat16)
        tg = tgtf[bs]                                    # (BPC, CH, P, NDG, G)
        oh = (tg[:, :, :, :OHH, :, None] ==
              np.arange(C)[None, None, None, None, None, :])
        # layout (d, k, g) per partition
        oh = oh.transpose(0, 1, 2, 3, 5, 4).reshape(BPC, CH, P, OHH * 128)
        oh = np.ascontiguousarray(oh).astype(ml_dtypes.float8_e4m3)
        td = tg[:, :, :, OHH:, :].transpose(2, 0, 1, 3, 4).reshape(
            P, BPC * CH * TOD)
        td = np.ascontiguousarray(td).astype(np.float32).astype(
            ml_dtypes.bfloat16)
        pk = np.concatenate([
            p8.view(np.uint8), pb.view(np.uint8),
            oh.view(np.uint8)], axis=-1)
        in_maps.append({"packed": np.ascontiguousarray(pk),
                        "tgt": td})

    res = bass_utils.run_bass_kernel_spmd(
        nc, in_maps, core_ids=list(range(N_CORES)), trace=TRACE)
    LAST_EXEC_NS = res.exec_time_ns
    LAST_TRACE = (res.instructions_and_trace[1]
                  if res.instructions_and_trace else None)

    # host combine (tiny): S[b,k,ci] = sum_g smat[k*16+g, g*8+ci]
    S = np.zeros((B, C, C), np.float64)
    total_lse = 0.0
    for core in range(N_CORES):
        out = res.results[core]["out"]
        # out[p=k*16+g, b*128 + gp*8 + ci] for the smat part
        smat = out[:, :2 * C * G].reshape(C, G, BPC, G, C)
        S[core * BPC:(core + 1) * BPC] = np.einsum(
            "kgbgc->bkc", smat.astype(np.float64))
        total_lse += out[:, 2 * C * G:].astype(np.float64).sum()

    n = np.zeros((B, C), np.float64)
    for b in range(B):
        n[b] = np.bincount(target[b].ravel().astype(np.int64), minlength=C)

    M = S.transpose(0, 2, 1) / n[:, None, :]             # M[b,ci,ck]
    diag = np.einsum("bcc->bc", M)
    inner = (diag[:, :, None] - M) * 0.5
    off = 1.0 - np.eye(C)
    jl = (-(np.log(0.5 + inner) * off).sum(axis=(1, 2))).mean()
    ce = (total_lse - np.einsum("bkk->", S)) / (B * N)
    return np.float32(jl + ce)



# revision 7
# speedup vs baseline: 1.3113x; 1.3113x over previous
"""J-regularized cross-entropy loss on 8 Trainium2 cores.

Per core (2 batches, N=262144 px, C=8): host sorts each batch's pixels
by target class and pads every class run to RC cols (32 px/col, zero
pixels), so the device program is data-independent. Layout: two SBUF
tiles (half h = classes 4h..4h+3), partition = 32*(c%4) + slot,
free = pixel column. All pred ships as fp8e3 (4.3 MB/core, one stream).

Device:
  codes[p, t] = int16(round(A*x + B))       (Schraudolph exp bits)
      DVE tensor_scalar fp8->int16 (2x mode) + ACT Copy scale/bias for
      a share of the columns.
  lse:  blockones [128,32] matmul pairs (halves accumulate) ->
      psum [32,512] regions, 4 col-tiled regions/bank; ACT Ln over
      [128,1024] psum with accum_out -> per-partition lse sums.
  S[b,k,c] = sum of class-k pred: ones4 [128,4] matmul over the raw
      fp8 pred tiles, output [4, L/2, 2] stride-(0,1) psum cells
      (free-dim accumulation via has_written), one cell pair per
      (run, half); zero-dummy MM pre-clears the bank.

Host: unpack S cells, lse total minus the (fixed 16384/core) zero-pad
pixels' ln(8*v0), then M = S^T/n, jl, ce exactly as the reference.
"""

import numpy as np
import ml_dtypes

import concourse.bacc as bacc
import concourse.mybir as mybir
import concourse.tile as tile
from concourse import bass_utils

N_CORES = 8
B, C, H, W = 16, 8, 512, 512
N = H * W
P = 128
BPC = B // N_CORES        # batches per core
SLOTS = 32                # pixels per column
CHUNK = 512               # cols per blockones matmul
NRUNS = BPC * C           # class runs per core (16)

LOG2E = 1.4426950408889634
SCHRAU_A = 128.0 * LOG2E
SCHRAU_B = 16256.0 - 7.368
V0 = 0.97265625           # int16 16249 viewed as bf16 (code of x=0)
ACT_SET_NL = 6            # natural_log_exp_and_others

NPIECE = 8                # DMA/code pieces per half
ACT_PIECES = (2, 6)       # piece indices handled by ACT (per half)

TRACE = False
LAST_EXEC_NS = None
LAST_TRACE = None

_BF16 = mybir.dt.bfloat16
_F32 = mybir.dt.float32
_F8E3 = mybir.dt.float8e3
_I16 = mybir.dt.int16

_nc_cache = {}


def _build_nc(RC):
    T = NRUNS * RC                      # cols per core (multiple of 512)
    NCHK = T // CHUNK
    PIECE = T // NPIECE                 # = 2*RC, multiple of 64
    NGRP = (NCHK + 7) // 8              # ln groups ([128,1024] psum tiles)
    extra_ln = 1 if (NCHK % 8) in (5, 6, 7) else 0
    NOUT = 16 + NGRP + extra_ln

    nc = bacc.Bacc("TRN2", target_bir_lowering=False, debug=False,
                   num_devices=N_CORES)
    pk_d = nc.dram_tensor("pk", (P, 2 * T), mybir.dt.uint8,
                          kind="ExternalInput")
    w_d = nc.dram_tensor("w", (P, 64), _BF16, kind="ExternalInput")
    out_d = nc.dram_tensor("out", (P, NOUT), _F32, kind="ExternalOutput")

    with tile.TileContext(nc) as tc:
        nc.scalar.add_instruction(mybir.InstLoadActFuncSet(
            name=nc.get_next_instruction_name(),
            act_func_set_id=ACT_SET_NL, ins=[], outs=[]))
        with (
            tc.tile_pool(name="big", bufs=1) as big,
            tc.tile_pool(name="lse", bufs=2, space="PSUM") as lse_pool,
            tc.tile_pool(name="s4", bufs=1, space="PSUM") as s4_pool,
        ):
            pk = big.tile([P, 2 * T], mybir.dt.uint8, name="pk")
            codes = big.tile([P, 2 * T], _I16, name="codes")
            w = big.tile([P, 64], _BF16, name="w")
            out_sb = big.tile([P, NOUT], _F32, name="out_sb")
            lnt = big.tile([P, 1024], _BF16, name="lnt")

            bo = w[:, 0:32]     # blockones: w[p, p%32] = 1
            o4 = w[:, 32:36]    # ones4: w[p, 32 + p//32] = 1
            zo = w[:, 36:37]    # zeros

            def p8(h, c0, c1):
                return pk[:, h * T + c0: h * T + c1].bitcast(_F8E3)

            def cb(h, c0, c1):
                return codes[:, h * T + c0: h * T + c1].bitcast(_BF16)

            nc.sync.dma_start(w[:, :], w_d[:, :])

            # input DMA, interleaving halves so early cols of both
            # halves land first
            for pc in range(NPIECE):
                for h in range(2):
                    lo = h * T + pc * PIECE
                    nc.sync.dma_start(pk[:, lo:lo + PIECE],
                                      pk_d[:, lo:lo + PIECE])

            # S accumulator bank: pre-clear via zero matmul
            s4 = s4_pool.tile([P, 16], _F32, name="s4")
            nc.tensor.matmul(s4[:, :], zo.broadcast_to([P, 128]),
                             w[:, 0:16], start=True, stop=False,
                             skip_group_check=True)

            # codes generation per piece
            for pc in range(NPIECE):
                for h in range(2):
                    lo = h * T + pc * PIECE
                    src = pk[:, lo:lo + PIECE].bitcast(_F8E3)
                    dst = codes[:, lo:lo + PIECE]
                    if pc in ACT_PIECES:
                        nc.scalar.activation(
                            dst, src, mybir.ActivationFunctionType.Copy,
                            bias=SCHRAU_B, scale=SCHRAU_A)
                    else:
                        nc.vector.tensor_scalar(
                            dst, src, SCHRAU_A, SCHRAU_B,
                            mybir.AluOpType.mult, mybir.AluOpType.add)

            # ones4 matmul slices per (run, half) unit
            def ones4_unit(r, h):
                u = r * 2 + h
                q = u % 4
                cp = u // 4
                cell = s4[32 * q:32 * q + 4, 2 * cp:2 * cp + 2]
                c0 = r * RC
                slices = []
                while c0 < (r + 1) * RC:
                    ln_ = min(512, (r + 1) * RC - c0)
                    slices.append((c0, c0 + ln_))
                    c0 += ln_
                for si, (a, b_) in enumerate(slices):
                    outap = cell.unsqueeze(1).broadcast_to(
                        [4, (b_ - a) // 2, 2])
                    nc.tensor.matmul(
                        outap, o4, p8(h, a, b_),
                        start=False, stop=(si == len(slices) - 1),
                        tile_position=(0, 32 * q),
                        skip_group_check=True)

            # blockones matmuls + ln per psum group
            for g in range(NGRP):
                ps = lse_pool.tile([P, 1024], _F32, tag="lse", name="ps")
                nch = min(8, NCHK - g * 8)
                for jj in range(nch):
                    j = g * 8 + jj
                    r = jj % 4
                    colh = jj // 4
                    reg = ps[32 * r:32 * r + 32,
                             512 * colh:512 * colh + 512]
                    a = j * CHUNK
                    nc.tensor.matmul(reg, bo, cb(0, a, a + CHUNK),
                                     start=True, stop=False,
                                     tile_position=(0, 32 * r))
                    nc.tensor.matmul(reg, bo, cb(1, a, a + CHUNK),
                                     start=False, stop=True,
                                     tile_position=(0, 32 * r))
                # interleave ones4 units whose data is ready around here
                for r in range(NRUNS):
                    if r * RC // (8 * CHUNK) == g:
                        ones4_unit(r, 0)
                        ones4_unit(r, 1)
                if nch == 8:
                    nc.scalar.activation(
                        lnt[:, 0:1024], ps[:, 0:1024],
                        mybir.ActivationFunctionType.Ln,
                        accum_out=out_sb[:, 16 + g:17 + g])
                elif nch <= 4:
                    nc.scalar.activation(
                        lnt[0:32 * nch, 0:512], ps[0:32 * nch, 0:512],
                        mybir.ActivationFunctionType.Ln,
                        accum_out=out_sb[0:32 * nch, 16 + g:17 + g])
                else:
                    nc.scalar.activation(
                        lnt[:, 0:512], ps[:, 0:512],
                        mybir.ActivationFunctionType.Ln,
                        accum_out=out_sb[:, 16 + g:17 + g])
                    nc.scalar.activation(
                        lnt[0:32 * (nch - 4), 512:1024],
                        ps[0:32 * (nch - 4), 512:1024],
                        mybir.ActivationFunctionType.Ln,
                        accum_out=out_sb[0:32 * (nch - 4),
                                         16 + NGRP:17 + NGRP])

            nc.vector.tensor_copy(out_sb[:, 0:16], s4[:, :])
            nc.sync.dma_start(out_d[:, :], out_sb[:, :],
                              single_packet=True)

    nc.compile()
    return nc


def kernel(pred, target):
    global LAST_EXEC_NS, LAST_TRACE
    pred = np.asarray(pred)
    target = np.asarray(target)

    pred8 = pred.astype(ml_dtypes.float8_e3m4)   # quantize once, full

    # per-batch class counts and sorted orders
    orders, counts = [], []
    for b in range(B):
        t = target[b].reshape(N).astype(np.int64)
        orders.append(np.argsort(t, kind="stable"))
        counts.append(np.bincount(t, minlength=C))
    counts = np.array(counts)                     # (B, C)

    RC = 32 * int(np.ceil(counts.max() / 1024.0))
    T = NRUNS * RC

    if RC not in _nc_cache:
        _nc_cache[RC] = _build_nc(RC)
    nc = _nc_cache[RC]

    wmat = np.zeros((P, 64), dtype=np.float32)
    for p in range(P):
        wmat[p, p % 32] = 1.0
        wmat[p, 32 + p // 32] = 1.0
    wmat = wmat.astype(ml_dtypes.bfloat16)

    in_maps = []
    for core in range(N_CORES):
        cols = []
        for bb in range(BPC):
            b = core * BPC + bb
            pb8 = pred8[b].reshape(C, N)
            z = np.zeros((C, 1), dtype=pb8.dtype)
            pbx = np.concatenate([pb8, z], axis=1)    # sentinel col
            idx = np.full((C, RC * 32), N, dtype=np.int64)
            ofs = 0
            for k in range(C):
                nk = counts[b, k]
                idx[k, :nk] = orders[b][ofs:ofs + nk]
                ofs += nk
            cols.append(pbx[:, idx.reshape(-1)])      # (C, 8*RC*32)
        full = np.concatenate(cols, axis=1)           # (C, T*32)
        arr = full.reshape(C, T, SLOTS)
        halves = []
        for h in range(2):
            a = arr[4 * h:4 * h + 4]                  # (4, T, 32)
            tilearr = a.transpose(0, 2, 1).reshape(P, T)
            halves.append(tilearr.view(np.uint8))
        pkarr = np.ascontiguousarray(
            np.concatenate(halves, axis=1))           # (128, 2T)
        in_maps.append({"pk": pkarr, "w": wmat})

    res = bass_utils.run_bass_kernel_spmd(
        nc, in_maps, core_ids=list(range(N_CORES)), trace=TRACE)
    LAST_EXEC_NS = res.exec_time_ns
    LAST_TRACE = (res.instructions_and_trace[1]
                  if res.instructions_and_trace else None)

    NCHK = T // CHUNK
    NGRP = (NCHK + 7) // 8

    S = np.zeros((B, C, C), np.float64)               # S[b, k, c]
    lse_dev = 0.0
    for core in range(N_CORES):
        out = res.results[core]["out"].astype(np.float64)
        for r in range(NRUNS):
            bb, k = divmod(r, C)
            b = core * BPC + bb
            for h in range(2):
                u = r * 2 + h
                q = u % 4
                cp = u // 4
                cell = out[32 * q:32 * q + 4, 2 * cp:2 * cp + 2]
                S[b, k, 4 * h:4 * h + 4] += cell.sum(axis=1)
        for g in range(NGRP):
            nch = min(8, NCHK - g * 8)
            if nch == 8:
                lse_dev += out[:, 16 + g].sum()
            elif nch <= 4:
                lse_dev += out[0:32 * nch, 16 + g].sum()
            else:
                lse_dev += out[:, 16 + g].sum()
                lse_dev += out[0:32 * (nch - 4), 16 + NGRP].sum()

    n_pads = N_CORES * (T * SLOTS - BPC * N)
    lse_real = lse_dev - n_pads * np.log(C * V0)

    n = counts.astype(np.float64)
    M = S.transpose(0, 2, 1) / n[:, None, :]          # M[b, c, k]
    diag = np.einsum("bcc->bc", M)
    inner = (diag[:, :, None] - M) * 0.5
    off = 1.0 - np.eye(C)
    jl = (-(np.log(0.5 + inner) * off).sum(axis=(1, 2))).mean()
    trS = np.einsum("bkk->", S)
    ce = (lse_real - trS) / (B * N)
    return np.float32(jl + ce)
